# revision 40
# baseline (speedup 1.0000x reference)
"""Trainium2 Bass kernel for a dense transformer encoder layer.

Problem: B=2, S=2048, D=1024, H=16 heads (W=64), F=4096, fp32.

Sharding: 8 cores = 2 batches x 4 sequence chunks of 512 tokens. Each core
computes K/V for its batch's full sequence (replicated within its 4-core
batch group) and Q/attention/FFN for its own 512-token chunk. No collectives.

Dataflow: activations live TRANSPOSED in SBUF ([feature, token], feature on
partitions) so QKV projections, attention, output projection and both FFN
matmuls chain on the TensorEngine with no on-device transposes. The host
transposes x on the way in and the per-core 1024x512 output on the way out.

Precision: every big matmul runs in fp8 DoubleRow mode (2 contraction
chunks per instruction, 0.5 PE cycles/row). Weights are pre-scaled by 64 on
the host so their ~0.02-sigma values land mid-range in e4m3; q,k carry an
8x slice of that scale into the scores matmul and exp() folds the rest.
Accuracy is recovered with residual-compensation chains accumulated into
the same PSUM: h1 and gelu(z) each get an e5m2 residual tensor, and w1 gets
an e5m2 residual weight (w1 = hi + lo exactly to ~0.1%). w2 stays a single
e4m3 tensor; its ~1.2e-2 contribution dominates the final error (~1.6x
under the 2e-2 gate). Residuals, LayerNorm and softmax statistics stay
fp32/f32r.

Softmax: score tiles are [key-token, query-token]. exp runs on two lanes,
software-pipelined 6 slots deep: ACT computes true exp into fp8, and DVE
computes a Schraudolph exp - i8 = round(11.54*t + 55.54) bitcast as e4m3
bit patterns (~8% rel err; numerator and Z share the values so it cancels
to first order, and attention is only ~1% of the layer output). The additive
-10000 mask folds multiplicatively into V and into the per-head Z column as
gamma/64, so Z's reciprocal row is 64/Z and hT lands unit-scale in fp8; the
row is broadcast via DMA-to-partition-0 + gpsimd partition_broadcast, off
the exp-critical engines.

LayerNorm: statistics ride 1/D-scaled ones-column matmuls; the affine apply
is dst = src*A - B against PE-built rank-1/rank-2 tiles (B in ONE K=2
matmul from [g;-b] x [u*rstd;ones]), with chunks distributed over
DVE/ACT/Pool. GPSIMD cannot touch PSUM, so evacuations live on DVE/ACT
(activation Copy/Identity with scale+bias APs) and Pool gets SBUF-only work.

Bias algebra (exact): bk drops out of softmax (constant per query row);
bv commutes through attention into bo' = bo + bv @ wo (host-folded).

SBUF overlays: the pre-LN1 residual x rides in h1f (dead before LN1 writes
h1); wo8 borrows g1T8's first half (dead before the first gelu). w2 is
loaded whole behind the streamed w1 groups so FFN2 runs DMA-free.
"""
import numpy as np
import concourse.bass as bass
from concourse import bacc
import concourse.mybir as mybir
import concourse.tile as tile
from concourse.bass import ts
from concourse.bass_utils import run_bass_kernel_spmd

P = 128
B, S, D, H, W, F = 2, 2048, 1024, 16, 64, 4096
DC = D // P            # 8 d-chunks
FC = F // P            # 32 f-chunks
TC = S // P            # 16 key-token chunks
SCH = 512              # tokens per core
EPS = 1e-12
SCALE = 1.0 / np.sqrt(np.float32(W))
WA = W + 1             # per-head V columns incl. ones column
WS = 64.0              # host-side fp8 weight pre-scale

F32 = mybir.dt.float32
DT = mybir.dt.float32r
F8 = mybir.dt.float8e4
F8L = mybir.dt.float8e5
DRM = mybir.MatmulPerfMode.DoubleRow

_cache = {}


def _layer_norm(nc, tc, pq, ppacc, invd, urow2, src, sq, dst,
                grow, gbrow, tag, dst8=None, dst8b=None):
    """src/sq/dst: [P, DC, SCH] sbuf (feature on partitions). LN over features.
    sq = src*src comes from the caller's producing evacuation. Mean scaling
    rides the stats matmuls via the invd column. The apply is two elementwise
    passes: dst = src*A - B with rank-1 A = g (x) rstd and rank-2
    B = g (x) u*rstd - b (x) 1, the latter built in ONE K=2 matmul from
    gbrow = [g; -b] rows and urow2 = [u*rstd; ones] rows. Chunks alternate
    DVE/Pool so neither engine serializes the apply. dst8, if given, gets an
    fp8 copy of dst (for the following fp8 matmul) on the opposite engine."""
    at = mybir.ActivationFunctionType
    with tc.tile_pool(name=tag, bufs=1) as pool:
        ps_u = pq.tile([1, SCH], F32, tag="st", bufs=2)
        ps_v = pq.tile([1, SCH], F32, tag="st", bufs=2)
        for dc in range(DC):
            nc.tensor.matmul(ps_u[:], invd[:], src[:, dc],
                             start=(dc == 0), stop=(dc == DC - 1))
        for dc in range(DC):
            nc.tensor.matmul(ps_v[:], invd[:], sq[:, dc],
                             start=(dc == 0), stop=(dc == DC - 1))
        var = pool.tile([1, SCH], F32)
        sd = pool.tile([1, SCH], F32)
        rstd = pool.tile([1, SCH], DT)
        nc.scalar.activation(var[:], ps_u[:], at.Square)
        nc.vector.tensor_tensor(var[:], ps_v[:], var[:], mybir.AluOpType.subtract)
        nc.scalar.activation(sd[:], var[:], at.Sqrt, bias=EPS)
        nc.vector.reciprocal(rstd[:], sd[:])
        nc.vector.tensor_tensor(urow2[0:1], ps_u[:], rstd[:],
                                mybir.AluOpType.mult)
        for dc in range(DC):
            ps_a = ppacc.tile([P, SCH], F32, tag="acc")
            ps_b = pq.tile([P, SCH], F32, tag="ps", bufs=4)
            nc.tensor.matmul(ps_a[:], grow[:, ts(dc, P)], rstd[:],
                             start=True, stop=True)
            nc.tensor.matmul(ps_b[:], gbrow[0:2, ts(dc, P)], urow2[0:2],
                             start=True, stop=True)
            t = pool.tile([P, SCH], F32, tag="lnt", bufs=4)
            if dc % 2 == 0:
                nc.vector.tensor_tensor(t[:], src[:, dc], ps_a[:],
                                        mybir.AluOpType.mult)
            else:
                # route around DVE: ACT evacuates A, Pool does the multiply
                sa = pool.tile([P, SCH], F32, tag="sa", bufs=2)
                nc.scalar.activation(sa[:], ps_a[:], at.Copy)
                nc.gpsimd.tensor_tensor(t[:], src[:, dc], sa[:],
                                        mybir.AluOpType.mult)
            nc.vector.tensor_tensor(dst[:, dc], t[:], ps_b[:],
                                    mybir.AluOpType.subtract)
            if dst8 is not None:
                nc.scalar.activation(dst8[:, dc], dst[:, dc], at.Copy)
                eng = nc.gpsimd if dc % 2 == 0 else nc.vector
                eng.tensor_tensor(dst8b[:, dc], dst[:, dc], dst8[:, dc],
                                  mybir.AluOpType.subtract)


def _build():
    at = mybir.ActivationFunctionType
    nc = bacc.Bacc("TRN2", target_bir_lowering=False)

    xT8_d = nc.dram_tensor("xT8", [P, DC, S], F8, kind="ExternalInput")
    xs8_d = nc.dram_tensor("xs8", [P, DC, SCH], F8, kind="ExternalInput")
    xs_d = nc.dram_tensor("xs", [P, DC, SCH], DT, kind="ExternalInput")
    wq_d = nc.dram_tensor("wq8", [P, DC, D], F8, kind="ExternalInput")
    wk_d = nc.dram_tensor("wk8", [P, DC, D], F8, kind="ExternalInput")
    wv_d = nc.dram_tensor("wv8", [P, DC, D], F8, kind="ExternalInput")
    wo_d = nc.dram_tensor("wo8", [P, DC, D], F8, kind="ExternalInput")
    w1_d = nc.dram_tensor("w18", [P, DC, F], F8, kind="ExternalInput")
    w1l_d = nc.dram_tensor("w18l", [P, DC, F], F8L, kind="ExternalInput")
    w2_d = nc.dram_tensor("w28", [P, FC, D], F8, kind="ExternalInput")
    bq_d = nc.dram_tensor("bq64", [P, DC], F32, kind="ExternalInput")
    bo_d = nc.dram_tensor("boP", [P, DC], F32, kind="ExternalInput")
    bf1_d = nc.dram_tensor("bf1", [P, FC], F32, kind="ExternalInput")
    bf2_d = nc.dram_tensor("bf2", [P, DC], F32, kind="ExternalInput")
    gam_d = nc.dram_tensor("gam64", [P, TC], F32, kind="ExternalInput")
    invd_d = nc.dram_tensor("invd", [P, 1], DT, kind="ExternalInput")
    g1r_d = nc.dram_tensor("g1r", [1, D], DT, kind="ExternalInput")
    g2r_d = nc.dram_tensor("g2r", [1, D], DT, kind="ExternalInput")
    gb1_d = nc.dram_tensor("gb1", [2, D], DT, kind="ExternalInput")
    gb2_d = nc.dram_tensor("gb2", [2, D], DT, kind="ExternalInput")
    gamh_d = nc.dram_tensor("gamh", [P, TC, H], DT, kind="ExternalInput")
    ones_d = nc.dram_tensor("ones_c", [P, 512], DT, kind="ExternalInput")
    out_d = nc.dram_tensor("outT", [P, DC, SCH], F32, kind="ExternalOutput")

    with nc.allow_low_precision(reason="fp8/f32r matmuls are rounded by design"), \
         tile.TileContext(nc) as tc:
        with tc.tile_pool(name="small", bufs=1) as small, \
             tc.tile_pool(name="psacc", bufs=2, space="PSUM") as ppacc:

            # ---- small constants ----
            bq_sb = small.tile([P, DC], F32)
            bo_sb = small.tile([P, DC], F32)
            bf1_sb = small.tile([P, FC], F32)
            bf2_sb = small.tile([P, DC], F32)
            gam_sb = small.tile([P, TC], F32)
            invd = small.tile([P, 1], DT)
            urow2 = small.tile([2, SCH], DT)
            epsc = small.tile([P, 1], F32)
            gelw = small.tile([1, 1], F32)
            nc.vector.memset(epsc[:], EPS)
            nc.const_aps.aps[(F32, EPS)] = epsc[:]

            # long-lived tiles, allocated in reverse order of death (LIFO pools)
            hT, hT_free = tc.tile([P, DC, SCH], F8, name="hT")
            # out-proj / LN1 / FFN prefetch tiles sit low on the stack (they
            # outlive attention); their DMAs are issued right before attention
            prow_cm = tc.tile_pool(name="prow", bufs=1)
            prow = prow_cm.__enter__()
            g1r = prow.tile([1, D], DT)
            gb1 = prow.tile([2, D], DT)
            pf1_cm = tc.tile_pool(name="pf1", bufs=3)
            pf1 = pf1_cm.__enter__()
            w1t0 = pf1.tile([P, DC, 4 * P], F8, tag="wt", name="w1t0")
            w1lt0 = pf1.tile([P, DC, 4 * P], F8L, tag="wtl", name="w1lt0")
            h1f, h1f_free = tc.tile([P, DC, SCH], DT, name="h1f")
            h18, h18_free = tc.tile([P, DC, SCH], F8, name="h18")
            h18b, h18b_free = tc.tile([P, DC, SCH], F8L, name="h18b")
            g8b, g8b_free = tc.tile([P, FC, SCH], F8L, name="g8b")
            r2T, r2T_free = tc.tile([P, DC, SCH], DT, name="r2T")
            g1T8, g1T8_free = tc.tile([P, FC, SCH], F8, name="g1T8")
            # overlays: the pre-LN1 residual x rides in h1f (dead before the
            # LN1 apply writes h1), and wo8 borrows g1T8's first half (dead
            # before the first gelu lands)
            xs2 = h1f
            wo8 = g1T8[:, 0:16, :].rearrange("p (a two) n -> p a (two n)",
                                             two=2)
            vA, vA_free = tc.tile([P, TC, H * WA], F8, name="vA")
            vA_h = vA[:].rearrange("p t (h c) -> p t h c", c=WA)
            kT, kT_free = tc.tile([P, DC, S], F8, name="kT")
            qT, qT_free = tc.tile([P, DC, SCH], F8, name="qT")
            xT8, xT8_free = tc.tile([P, DC, S], F8, name="xT8")

            gamh_sb = small.tile([P, TC, H], DT)

            # ================= Phase V =================
            # v stored [token, feature] with a ones column per head (for Z).
            with tc.tile_pool(name="pv", bufs=1) as pv, \
                 tc.tile_pool(name="pvq", bufs=4, space="PSUM") as pvq:
                wv8 = pv.tile([P, DC, D], F8)
                wk8 = pv.tile([P, DC, D], F8)
                wq8 = pv.tile([P, DC, D], F8)
                xs8 = pv.tile([P, DC, SCH], F8)
                # DMA transfers serialize globally in the cost model: strict
                # first-needed-first order. x first half + wv first half gate
                # the first matmul; everything small queues after.
                nc.sync.dma_start(xT8[:, :, 0:512], xT8_d[:, :, 0:512])
                nc.scalar.dma_start(wv8[:, :, 0:512], wv_d[:, :, 0:512])
                nc.sync.dma_start(xT8[:, :, 512:1024], xT8_d[:, :, 512:1024])
                nc.sync.dma_start(gam_sb[:], gam_d[:])
                nc.scalar.dma_start(wv8[:, :, 512:], wv_d[:, :, 512:])
                nc.sync.dma_start(xT8[:, :, 1024:], xT8_d[:, :, 1024:])
                nc.scalar.dma_start(wk8[:], wk_d[:])
                nc.sync.dma_start(wq8[:], wq_d[:])
                nc.scalar.dma_start(xs8[:], xs8_d[:])
                nc.sync.dma_start(invd[:], invd_d[:])
                # gamma column per head (Z weights; = mask gamma / 64)
                nc.sync.dma_start(gamh_sb[:], gamh_d[:])
                nc.vector.tensor_copy(vA_h[:, :, :, W], gamh_sb[:])
                for sb, dr in [(bq_sb, bq_d), (bo_sb, bo_d),
                               (bf1_sb, bf1_d), (bf2_sb, bf2_d)]:
                    nc.sync.dma_start(sb[:], dr[:])
                for tcl in range(TC):
                    for dvh in range(2):
                        psv = pvq.tile([P, 512], F32, tag="ps", name="psv")
                        for i in range(DC // 2):
                            nc.tensor.matmul(psv[:],
                                             xT8[:, 2 * i:2 * i + 2, ts(tcl, P)],
                                             wv8[:, 2 * i:2 * i + 2, ts(dvh, 512)],
                                             start=(i == 0), stop=(i == DC // 2 - 1),
                                             perf_mode=DRM)
                        # gpsimd cannot touch PSUM: split evacuations DVE/ACT
                        if dvh == 0:
                            nc.vector.tensor_scalar(
                                vA_h[:, tcl, 0:8, 0:W],
                                psv[:].rearrange("p (h c) -> p h c", c=W),
                                gam_sb[:, tcl:tcl + 1], None,
                                mybir.AluOpType.mult,
                            )
                        else:
                            nc.scalar.activation(
                                vA_h[:, tcl, 8:16, 0:W],
                                psv[:].rearrange("p (h c) -> p h c", c=W),
                                at.Copy, scale=gam_sb[:, tcl:tcl + 1],
                            )

                # ================= Phase K =================
                # kT stored [feature, token], carrying the 64x weight scale.
                for tw in range(S // 512):
                    for dk in range(DC):
                        psk = pvq.tile([P, 512], F32, tag="ps", name="psk")
                        for i in range(DC // 2):
                            nc.tensor.matmul(psk[:],
                                             wk8[:, 2 * i:2 * i + 2, ts(dk, P)],
                                             xT8[:, 2 * i:2 * i + 2, ts(tw, 512)],
                                             start=(i == 0), stop=(i == DC // 2 - 1),
                                             perf_mode=DRM)
                        if dk % 2 == 0:
                            nc.vector.tensor_scalar(kT[:, dk, ts(tw, 512)],
                                                    psk[:], 0.125, None,
                                                    mybir.AluOpType.mult)
                        else:
                            nc.scalar.activation(kT[:, dk, ts(tw, 512)], psk[:],
                                                 at.Copy, scale=0.125)

                # ================= Phase Q =================
                # qT carries the 64x weight scale (bq pre-scaled to match).
                for dq in range(DC):
                    psq = pvq.tile([P, SCH], F32, tag="ps", name="psq")
                    for i in range(DC // 2):
                        nc.tensor.matmul(psq[:],
                                         wq8[:, 2 * i:2 * i + 2, ts(dq, P)],
                                         xs8[:, 2 * i:2 * i + 2],
                                         start=(i == 0), stop=(i == DC // 2 - 1),
                                         perf_mode=DRM)
                    if dq % 2 == 0:
                        nc.vector.tensor_scalar(qT[:, dq], psq[:], 0.125,
                                                bq_sb[:, dq:dq + 1],
                                                mybir.AluOpType.mult,
                                                mybir.AluOpType.add)
                    else:
                        nc.scalar.activation(qT[:, dq], psq[:], at.Identity,
                                             scale=0.125,
                                             bias=bq_sb[:, dq:dq + 1])
            xT8_free()

            # ---- prefetch transfers: land while attention runs ----
            # urow2 row 1 stays ones; row 0 is rewritten by each LN
            nc.sync.dma_start(urow2[0:2], ones_d[0:2, 0:SCH])
            nc.sync.dma_start(wo8[:, 0:4], wo_d[:, 0:4])
            nc.sync.dma_start(wo8[:, 4:8], wo_d[:, 4:8])
            nc.sync.dma_start(xs2[:, 0:4], xs_d[:, 0:4])
            nc.sync.dma_start(xs2[:, 4:8], xs_d[:, 4:8])
            
            nc.sync.dma_start(w1t0[:], w1_d[:, :, ts(0, 4 * P)])
            nc.sync.dma_start(w1lt0[:], w1l_d[:, :, ts(0, 4 * P)])
            nc.sync.dma_start(g1r[:], g1r_d[:])
            nc.sync.dma_start(gb1[:], gb1_d[:])

            # ================= Attention =================
            # scores psum = (8q)·(8k) = 64*s; exp folds SCALE/64; probs fp8.
            # Exp tiles split ~ACT 9/16 : DVE 7/16 -- the DVE path is a
            # Schraudolph bitcast exp (int32 affine, reinterpret as f32)
            # with Pool downcasting to fp8. Z rides the gamma/64 column so
            # the reciprocal row is 64/Z and hT lands unit-scale in fp8.
            EXA8 = 11.541560327111708    # 8 / ln 2
            EXB8 = 55.5368                # 7*8 - 8*0.0579 (Schraudolph bias)
            # exp runs on three lanes: ACT (true exp), DVE and Pool
            # (Schraudolph: i8 = round(A*t + B) bitcast as e4m3, ~8% rel err;
            # numerator and Z share the values so it cancels to first order,
            # and attention is ~1% of the layer output). Pool cannot read
            # PSUM, so its lane gets the scores via a DMA PSUM->SBUF copy
            # (the DMA track idles during attention).
            DSET = {1, 3, 5, 7, 9, 11, 13, 17, 19, 21, 25, 27, 29}
            with tc.tile_pool(name="pat", bufs=1) as pat, \
                 tc.tile_pool(name="pats", bufs=3, space="PSUM") as pats:
                slots = [(h, tcp) for h in range(H) for tcp in range(TC // 2)]
                probs_of = {}

                def scores_exp(idx):
                    h, tcp = slots[idx]
                    hc, hp = h // 2, W * (h % 2)
                    pss = pats.tile([P, 2 * SCH], F32, tag="ps2", name="pss")
                    for j in range(2):
                        nc.tensor.matmul(pss[:, ts(j, SCH)],
                                         kT[hp:hp + W, hc, ts(2 * tcp + j, P)],
                                         qT[hp:hp + W, hc],
                                         start=True, stop=True)
                    if idx % 32 in DSET and idx < 120:
                        pi = pat.tile([P, 2 * SCH], mybir.dt.int8,
                                      tag="pi", bufs=6)
                        nc.vector.tensor_scalar(
                            pi[:], pss[:], float(EXA8 * SCALE / WS), EXB8,
                            mybir.AluOpType.mult, mybir.AluOpType.add)
                        probs_of[idx] = pi[:].bitcast(F8)
                    else:
                        probs = pat.tile([P, 2 * SCH], F8, tag="probs", bufs=7)
                        nc.scalar.activation(probs[:], pss[:], at.Exp,
                                             scale=float(SCALE / WS))
                        probs_of[idx] = probs[:]

                for k in range(6):
                    scores_exp(k)
                pso = None
                for idx, (h, tcp) in enumerate(slots):
                    if idx + 6 < len(slots):
                        scores_exp(idx + 6)
                    if tcp == 0:
                        pso = ppacc.tile([WA, SCH], F32, tag="acc", name="pso")
                    pr = probs_of.pop(idx)
                    last = tcp == TC // 2 - 1
                    nc.tensor.matmul(
                        pso[:],
                        vA[:, 2 * tcp:2 * tcp + 2, h * WA:(h + 1) * WA],
                        pr.rearrange("p (i n) -> p i n", i=2),
                        start=(tcp == 0), stop=last,
                        perf_mode=DRM)
                    if not last:
                        continue
                    hc, hp = h // 2, W * (h % 2)
                    rz = pat.tile([P, SCH], DT, tag="rz", bufs=2)
                    nc.vector.reciprocal(rz[W:W + 1], pso[W:W + 1])
                    # broadcast the 64/Z row to 64 partitions off the critical
                    # engines: DMA it to partition 0, Pool partition-broadcast
                    rz0 = pat.tile([1, SCH], DT, tag="rz0", bufs=2)
                    nc.sync.dma_start(rz0[:], rz[W:W + 1])
                    rzb = pat.tile([W, SCH], DT, tag="rzb", bufs=2)
                    nc.gpsimd.partition_broadcast(rzb[:], rz0[:], channels=W)
                    if hp == 0:
                        nc.vector.tensor_tensor(hT[0:W, hc], pso[0:W], rzb[:],
                                                mybir.AluOpType.mult)
                    else:
                        tn = pat.tile([W, SCH], F8, tag="ntmp", bufs=2)
                        nc.vector.tensor_tensor(tn[:], pso[0:W], rzb[:],
                                                mybir.AluOpType.mult)
                        nc.sync.dma_start(hT[hp:hp + W, hc], tn[:])
            qT_free()
            kT_free()
            vA_free()

            # ================= Out-proj + residual =================
            pq2_cm = tc.tile_pool(name="pq2", bufs=4, space="PSUM")
            pq2 = pq2_cm.__enter__()
            r1T, r1T_free = tc.tile([P, DC, SCH], DT, name="r1T")
            sq1, sq1_free = tc.tile([P, DC, SCH], DT, name="sq1")
            if True:
                # warm the Sqrt table before LN1 needs it
                nc.scalar.activation(gelw[:], epsc[0:1, :], at.Sqrt)
                for dp in range(DC):
                    psr = (ppacc.tile([P, SCH], F32, tag="acc", name="psr")
                           if dp < 2 else
                           pq2.tile([P, SCH], F32, tag="ps", name="psr"))
                    for i in range(DC // 2):
                        nc.tensor.matmul(psr[:],
                                         wo8[:, 2 * i:2 * i + 2, ts(dp, P)],
                                         hT[:, 2 * i:2 * i + 2],
                                         start=(i == 0), stop=(i == DC // 2 - 1),
                                         perf_mode=DRM)
                    nc.scalar.activation(r1T[:, dp], psr[:], at.Identity,
                                         scale=float(1.0 / (WS * WS)),
                                         bias=bo_sb[:, dp:dp + 1])
                    nc.vector.tensor_tensor(r1T[:, dp], r1T[:, dp], xs2[:, dp],
                                            mybir.AluOpType.add)
                    if dp % 2 == 0:
                        nc.scalar.activation(sq1[:, dp], r1T[:, dp], at.Square)
                    else:
                        nc.gpsimd.tensor_tensor(sq1[:, dp], r1T[:, dp],
                                                r1T[:, dp],
                                                mybir.AluOpType.mult)

            # ================= LN1 =================
            _layer_norm(nc, tc, pq2, ppacc, invd, urow2, r1T, sq1, h1f,
                        g1r, gb1, "ln1", dst8=h18, dst8b=h18b)
            sq1_free()
            r1T_free()
            # reuse the row tiles for LN2's affine rows; warm the Gelu table
            # while the LN1 apply is still draining on DVE/Pool
            nc.scalar.activation(gelw[:], epsc[0:1, :], at.Gelu)
            nc.scalar.dma_start(g1r[:], g2r_d[:])
            nc.scalar.dma_start(gb1[:], gb2_d[:])
            w28sb, w28sb_free = tc.tile([P, FC, D], F8, name="w28sb")
            sq2, sq2_free = tc.tile([P, DC, SCH], DT, name="sq2")

            # ================= FFN =================
            with tc.tile_pool(name="pgs", bufs=4) as pgs:
                def ffn1_chains(psg, w1t, w1lt, j, i, nchunks):
                    nc.tensor.matmul(psg[:], w1t[:, 2 * i:2 * i + 2, ts(j, P)],
                                     h18[:, 2 * i:2 * i + 2],
                                     start=(i == 0), stop=False, perf_mode=DRM)
                    nc.tensor.matmul(psg[:], w1t[:, 2 * i:2 * i + 2, ts(j, P)],
                                     h18b[:, 2 * i:2 * i + 2],
                                     start=False, stop=False, perf_mode=DRM)
                    nc.tensor.matmul(psg[:], w1lt[:, 2 * i:2 * i + 2, ts(j, P)],
                                     h18[:, 2 * i:2 * i + 2],
                                     start=False, stop=(i == nchunks - 1),
                                     perf_mode=DRM)

                def ffn1_gelu(psg, fc):
                    gf = pgs.tile([P, SCH], DT, tag="gf")
                    nc.scalar.activation(gf[:], psg[:], at.Gelu,
                                         bias=bf1_sb[:, fc:fc + 1],
                                         scale=float(1.0 / WS))
                    nc.gpsimd.tensor_copy(g1T8[:, fc], gf[:])
                    nc.vector.tensor_tensor(g8b[:, fc], gf[:], g1T8[:, fc],
                                            mybir.AluOpType.subtract)

                for fcg in range(FC // 4):
                    if fcg == 0:
                        # chunk-pair-major emission: the 4 chains fill as the
                        # LN1 apply streams h18/h18b chunk pairs out
                        psgs = [pq2.tile([P, SCH], F32, tag="ps", name="psg")
                                for _ in range(4)]
                        for i in range(DC // 2):
                            for j in range(4):
                                ffn1_chains(psgs[j], w1t0, w1lt0, j, i, DC // 2)
                        for j in range(4):
                            ffn1_gelu(psgs[j], j)
                        continue
                    w1t = pf1.tile([P, DC, 4 * P], F8, tag="wt")
                    w1lt = pf1.tile([P, DC, 4 * P], F8L, tag="wtl")
                    nc.sync.dma_start(w1t[:], w1_d[:, :, ts(fcg, 4 * P)])
                    nc.scalar.dma_start(w1lt[:], w1l_d[:, :, ts(fcg, 4 * P)])
                    if fcg % 2 == 1:
                        # w2 rides in behind the w1 streams so FFN2 is DMA-free
                        i = fcg // 2
                        nc.sync.dma_start(w28sb[:, ts(i, 8)], w2_d[:, ts(i, 8)])
                    for j in range(4):
                        fc = 4 * fcg + j
                        psg = pq2.tile([P, SCH], F32, tag="ps", name="psg")
                        for i in range(DC // 2):
                            ffn1_chains(psg, w1t, w1lt, j, i, DC // 2)
                        ffn1_gelu(psg, fc)
                sqwarm2 = pgs.tile([1, 1], F32, tag="warm")
                nc.scalar.activation(sqwarm2[:], epsc[0:1, :], at.Sqrt)
                for dp in range(DC):
                    psf = pq2.tile([P, SCH], F32, tag="ps", name="psf")
                    for i in range(FC // 2):
                        nc.tensor.matmul(psf[:],
                                         w28sb[:, 2 * i:2 * i + 2, ts(dp, P)],
                                         g1T8[:, 2 * i:2 * i + 2],
                                         start=(i == 0), stop=False,
                                         perf_mode=DRM)
                    for i in range(FC // 2):
                        nc.tensor.matmul(psf[:],
                                         w28sb[:, 2 * i:2 * i + 2, ts(dp, P)],
                                         g8b[:, 2 * i:2 * i + 2],
                                         start=False, stop=(i == FC // 2 - 1),
                                         perf_mode=DRM)
                    if dp % 2 == 0:
                        nc.vector.tensor_scalar(r2T[:, dp], psf[:],
                                                float(1.0 / WS),
                                                bf2_sb[:, dp:dp + 1],
                                                mybir.AluOpType.mult,
                                                mybir.AluOpType.add)
                    else:
                        nc.scalar.activation(r2T[:, dp], psf[:], at.Identity,
                                             scale=float(1.0 / WS),
                                             bias=bf2_sb[:, dp:dp + 1])
                    nc.gpsimd.tensor_tensor(r2T[:, dp], r2T[:, dp], h1f[:, dp],
                                            mybir.AluOpType.add)
                    nc.vector.tensor_tensor(sq2[:, dp], r2T[:, dp], r2T[:, dp],
                                            mybir.AluOpType.mult)
            # ================= LN2 + out =================
            oT, oT_free = tc.tile([P, DC, SCH], F32, name="oT")
            _layer_norm(nc, tc, pq2, ppacc, invd, urow2, r2T, sq2, oT,
                        g1r, gb1, "ln2")
            for i in range(4):
                eng = nc.sync if i % 2 == 0 else nc.scalar
                eng.dma_start(out_d[:, 2 * i:2 * i + 2], oT[:, 2 * i:2 * i + 2])
            oT_free()
            sq2_free()
            w28sb_free()
            pq2_cm.__exit__(None, None, None)
            g1T8_free()
            r2T_free()
            g8b_free()
            h18b_free()
            h18_free()
            h1f_free()
            pf1_cm.__exit__(None, None, None)
            prow_cm.__exit__(None, None, None)
            hT_free()

    nc.compile()
    return nc


def kernel(**inputs):
    x = np.asarray(inputs["x"], dtype=np.float32)
    mask = np.asarray(inputs["mask"])
    f = {k: np.asarray(inputs[k], dtype=np.float32) for k in
         ["wq", "bq", "wk", "bk", "wv", "bv", "wo", "bo", "g1", "b1",
          "w1", "bf1", "w2", "bf2", "g2", "b2"]}

    if "nc" not in _cache:
        _cache["nc"] = _build()
    nc = _cache["nc"]

    f8 = mybir.dt.np(F8)
    f8l = mybir.dt.np(F8L)

    def wlay8(w, pc):  # [K, M] -> [P, K//P, M], fp8 with 64x pre-scale
        a = (w * WS).astype(f8)
        return np.ascontiguousarray(a.reshape(pc, P, w.shape[1]).transpose(1, 0, 2))

    def wlay8lo(w, pc):  # e5m2 residual of the e4m3 hi part (same 64x scale)
        hi = (w * WS).astype(f8).astype(np.float32)
        a = (w * WS - hi).astype(f8l)
        return np.ascontiguousarray(a.reshape(pc, P, w.shape[1]).transpose(1, 0, 2))

    def blay(b):      # [M] -> [P, M//P]
        return np.ascontiguousarray(b.reshape(-1, P).T)

    shared = {
        "wq8": wlay8(f["wq"], DC), "wk8": wlay8(f["wk"], DC),
        "wv8": wlay8(f["wv"], DC), "wo8": wlay8(f["wo"], DC),
        "w18": wlay8(f["w1"], DC), "w28": wlay8(f["w2"], FC),
        "w18l": wlay8lo(f["w1"], DC),
        "invd": np.full((P, 1), 1.0 / D, np.float32),
        "g1r": f["g1"].reshape(1, D), "g2r": f["g2"].reshape(1, D),
        "gb1": np.stack([f["g1"], -f["b1"]]),
        "gb2": np.stack([f["g2"], -f["b2"]]),
        "bq64": blay(f["bq"]) * np.float32(8.0),
        "boP": blay(f["bo"] + f["bv"] @ f["wo"]),
        "bf1": blay(f["bf1"]), "bf2": blay(f["bf2"]),
        "ones_c": np.ones((P, 512), np.float32),
    }

    in_maps = []
    for c in range(8):
        b, sq = c // 4, c % 4
        xTb = np.ascontiguousarray(x[b].T.reshape(DC, P, S).transpose(1, 0, 2))
        xT8 = xTb.astype(f8)
        mbias = (-10000.0 * (1.0 - mask[b].astype(np.float32))).reshape(TC, P).T
        m = dict(shared)
        m["xT8"] = xT8
        m["xs8"] = np.ascontiguousarray(xT8[:, :, sq * SCH:(sq + 1) * SCH])
        m["xs"] = np.ascontiguousarray(xTb[:, :, sq * SCH:(sq + 1) * SCH])
        gam = np.exp(mbias).astype(np.float32)          # 1.0 unmasked, 0.0 masked
        m["gam64"] = np.ascontiguousarray(gam / np.float32(WS))
        m["gamh"] = np.ascontiguousarray(
            np.broadcast_to(gam[:, :, None] / np.float32(WS), (P, TC, H)))
        in_maps.append(m)

    res = run_bass_kernel_spmd(nc, in_maps, core_ids=list(range(8)))
    _cache["last_res"] = res

    out = np.empty((B, S, D), np.float32)
    for c in range(8):
        b, sq = c // 4, c % 4
        oT = res.results[c]["outT"]  # [P, DC, SCH]
        out[b, sq * SCH:(sq + 1) * SCH, :] = oT.transpose(2, 1, 0).reshape(SCH, D)
    return out
